# revision 41
# baseline (speedup 1.0000x reference)
"""DHEL contrastive loss kernel for Trainium2 (8 NeuronCores, SPMD).

Math (reference):
  zhat = z / max(||z||, 1e-12) rowwise;  za = zhat[:8192], zp = zhat[8192:]
  sa_i = sum_j!=i exp(za_i . za_j / tau);  sp_i = sum_j!=i exp(zp_i . zp_j / tau)
  pdot_i = za_i . zp_i
  loss = mean_i( log sa_i + log sp_i - pdot_i / tau )

Key idea vs the naive row-parallel split: the two similarity matrices are
SYMMETRIC, so only the upper triangle of 1024x1024 blocks needs the (scalar-
engine-bound) exp evaluation. Each computed off-diagonal block (R, C)
contributes its row-sums to rows R (free via the activation accumulator) and
its column-sums to rows C (bf16 tree-add on DVE + a partition-reduce on the
otherwise idle Pool engine). Per-core exp work drops from 2*8192^2/8 = 16.8M
elements to 9 blocks = 9.4M, and the scalar engine runs wide (2048-col)
back-to-back exp+accumulate over the whole kernel.

Work assignment: a tournament orientation of K8 gives every core exactly 9
blocks: its own diagonal block in each half, its in-star pairs in one half and
out-star pairs in the other (4 + 3). Cores 0-3 get [5 anchor blocks, 4
positive], cores 4-7 the mirror image; the device program is identical
("X half" = 5 column blocks, "Y half" = 4) and the host maps halves/blocks
per core and inverts the mapping when assembling.

Division of labor (per the sharding hint, devices consume all-gathered
NORMALIZED embeddings): the host normalizes rows in f64, casts to bf16, and
ships each core the transposed [d, columns] slice it contracts against
(2.3 MB/core); block-granular DMAs so the first matmul issues ~4 us in. The
O(N^2 d) similarity/exp/reduction work all happens on device. The host folds
the returned row/column partial sums, subtracts the (exactly reproducible)
bf16 self-similarity, adds the positive-pair dots, and takes log+mean in f64
-- O(N d) assembly, 0.03% of the FLOPs.
"""

import sys

if "/opt/trn_rl_repo" not in sys.path:
    sys.path.insert(0, "/opt/trn_rl_repo")

from contextlib import ExitStack

import numpy as np

import concourse.bass as bass  # noqa: F401
import concourse.tile as tile
from concourse import bacc, mybir
from concourse.bass_utils import run_bass_kernel_spmd

P = 128
D = 128
M = 16384
HALF = M // 2       # 8192
Q = 1024            # rows per block
NCORES = 8
NT = 72             # 9 blocks x 8 tiles of 128 rows
TAU = 0.3
SCALE = float(1.0 / TAU)

# column groups: (lhs base col, rhs col offset, width, [colacc ids], tri)
# Local zt column blocks: [Cx, x1..x4, Cy, y1..y3] at 1024 cols each.
# colacc ids 0..3 are X partners x1..x4, 4..6 are Y partners y1..y3; diag
# slices have no block colacc. The Cy+y1 group runs LAST so the tail is
# short. tri=True groups start at the diagonal block: row-chunk rc only
# computes columns >= rc*128 (the block is symmetric, the lower triangle is
# recovered as column sums -> colout slots 8 (Cx) / 9 (Cy) cover block
# columns 128..1024 from the chunks above them).
GROUPS = [
    (0, 0, 1024, [], True),             # Cx (diag)
    (0, 1024, 2048, [0, 1], False),     # x1 x2
    (0, 3072, 2048, [2, 3], False),     # x3 x4
    (5120, 7168, 2048, [5, 6], False),  # y2 y3
    (5120, 5120, 2048, [-1, 4], True),  # Cy (diag) + y1
]
# DMA arrival order for the 9 column blocks (group consumption order).
DMA_ORDER = (0, 1, 2, 3, 4, 5, 7, 8, 6)

F32 = mybir.dt.float32
BF16 = mybir.dt.bfloat16
AF = mybir.ActivationFunctionType
OP = mybir.AluOpType
AX = mybir.AxisListType


def _pair_owner(i: int, j: int) -> int:
    """Tournament owner of pair {i,j}, i<j: in-degrees (4,4,4,4,3,3,3,3)."""
    if j < 7:
        return i if (j - i) % 7 in (1, 2, 3) else j
    return i if i <= 3 else 7


def _core_layout(c: int):
    """Returns (x_is_anchor, xblocks, yblocks): block ids of the X (5-block)
    and Y (4-block) halves, center first."""
    a_star = []   # pairs {c,x} owned by c -> anchor-half partners
    p_star = []   # pairs not owned by c  -> positive-half partners
    for x in range(8):
        if x == c:
            continue
        i, j = min(c, x), max(c, x)
        (a_star if _pair_owner(i, j) == c else p_star).append(x)
    if len(a_star) == 4:
        return True, [c] + a_star, [c] + p_star
    return False, [c] + p_star, [c] + a_star


def _build(ctx: ExitStack, tc: tile.TileContext, zc_ext, rows_ext, cols_ext,
           etail_ext):
    nc = tc.nc

    persist = ctx.enter_context(tc.tile_pool(name="persist", bufs=1))
    eo_pool = ctx.enter_context(tc.tile_pool(name="eo", bufs=8))
    psum_pool = ctx.enter_context(tc.tile_pool(name="psum", bufs=2,
                                               space="PSUM"))

    zt = persist.tile([P, NT * P], BF16, name="zt")
    rows = persist.tile([P, 5, 8], F32, name="rows")
    colacc = [persist.tile([P, Q], BF16, name=f"colacc{b}") for b in range(7)]
    colaccD = [persist.tile([P, 896], BF16, name=f"colaccD{m}")
               for m in range(2)]
    colout = persist.tile([1, 10, Q], F32, name="colout")
    for k in DMA_ORDER:
        nc.sync.dma_start(zt[:, k * Q : (k + 1) * Q],
                          zc_ext[:, k * Q : (k + 1) * Q])

    def colsum_out(slot, src, off=0, width=Q):
        nc.gpsimd.tensor_reduce(colout[:, slot, off : off + width], src,
                                axis=AX.C, op=OP.add)
        nc.sync.dma_start(
            cols_ext[slot * Q + off : slot * Q + off + width].rearrange(
                "(o j) -> o j", o=1),
            colout[:, slot, off : off + width],
        )

    def sweep(g, posts=()):
        posts = list(posts)
        lhs0, off, w, caccs, tri = GROUPS[g]
        last = g == len(GROUPS) - 1
        # (a Pool bit-trick exp offload of tail columns was tried here and
        # reverted: the DVE row-sum + accumulate per offloaded chunk costs
        # more than the ACT time it saves under the scheduler)
        dacc = colaccD[0 if g == 0 else 1]
        for rc in range(8):
            cut = 128 * rc if tri else 0
            wr = w - cut
            aw = wr
            ps = psum_pool.tile([P, wr], F32, tag="ps", name=f"ps_g{g}_{rc}")
            col = 0
            while col < wr:
                cw = min(512, wr - col)
                nc.tensor.matmul(
                    ps[:, col : col + cw],
                    zt[:, lhs0 + rc * P : lhs0 + (rc + 1) * P],
                    zt[:, off + cut + col : off + cut + col + cw],
                    start=True, stop=True,
                )
                col += cw
            eo = eo_pool.tile([P, aw], BF16, tag="eo", name=f"eo_g{g}_{rc}")
            nc.scalar.activation(
                eo[:], ps[:, 0:aw], AF.Exp, scale=SCALE,
                accum_out=rows[:, g, rc : rc + 1],
            )
            if tri and rc < 7:
                # below-diagonal coverage of the diag block: columns
                # [(rc+1)*128, 1024) accumulate as column sums
                sl = eo[:, 128 : Q - cut]
                dst = dacc[:, rc * 128 : 896]
                if rc == 0:
                    nc.vector.tensor_copy(dst, sl)
                else:
                    nc.vector.tensor_tensor(dst, dst, sl, op=OP.add)
            for i, b in enumerate(caccs):
                if b < 0:
                    continue
                sl = eo[:, i * Q - cut : min((i + 1) * Q - cut, aw)]
                cw = sl.shape[1]
                if last and rc >= 6:
                    # tail: ship the raw bf16 chunk to DRAM; the host sums
                    # the 128 partitions -- no post-activation engine work.
                    a = rc - 6
                    nc.scalar.dma_start(etail_ext[a * P : (a + 1) * P, :], sl)
                elif rc == 0:
                    nc.vector.tensor_copy(colacc[b][:, 0:cw], sl)
                else:
                    nc.vector.tensor_tensor(colacc[b][:, 0:cw],
                                            colacc[b][:, 0:cw], sl,
                                            op=OP.add)
            if rc >= 1 and posts:
                posts.pop(0)()
            if last and rc == 5:
                # pre-reduce everything the tail depends on while chunks 6/7
                # still run: y1's rc0-5 partial and the diag columns below
                # 768 (rc6's add only touches 768..896)
                colsum_out(caccs[1], colacc[caccs[1]][:])
                colsum_out(9, dacc[:, 0:768], 0, 768)
            if last and rc == 6:
                colsum_out(9, dacc[:, 768:896], 768, 128)
        for t in posts:
            t()
        pending = []
        if not last:
            if tri:
                pending.append(lambda: colsum_out(8, dacc[:], 0, 896))
            for b in caccs:
                if b >= 0:
                    pending.append(lambda b=b: colsum_out(b, colacc[b][:]))
        return pending

    pending = []
    for g in range(len(GROUPS)):
        pending = sweep(g, pending)

    nc.scalar.dma_start(rows_ext.rearrange("p (g r) -> p g r", g=5),
                        rows[:])


def build_kernel() -> bass.Bass:
    nc = bacc.Bacc("TRN2", target_bir_lowering=False, debug=False,
                   num_devices=NCORES)
    zc_ext = nc.dram_tensor("zc", (D, NT * P), BF16, kind="ExternalInput").ap()
    rows_ext = nc.dram_tensor("rows", (P, 40), F32, kind="ExternalOutput").ap()
    # cols slots 0..6: colacc column sums (slot 4 = the tail block's rc0-5
    # partial); slots 8/9: the Cx/Cy diag blocks' below-diagonal column sums
    # (block columns 128..1024, width 896). Slot 7 is reserved/unused.
    cols_ext = nc.dram_tensor("cols", (10 * Q,), F32,
                              kind="ExternalOutput").ap()
    # raw bf16 eo chunks 6/7 of the tail block's y1 slice (host sums them)
    etail_ext = nc.dram_tensor("etail", (2 * P, Q), BF16,
                               kind="ExternalOutput").ap()
    with tile.TileContext(nc) as tc:
        with ExitStack() as ctx:
            _build(ctx, tc, zc_ext, rows_ext, cols_ext, etail_ext)
    nc.compile()
    return nc


def _normalized_bf16(z: np.ndarray) -> np.ndarray:
    import ml_dtypes

    zf = np.asarray(z, dtype=np.float64)
    zf = zf / np.maximum(np.linalg.norm(zf, axis=1, keepdims=True), 1e-12)
    return zf.astype(ml_dtypes.bfloat16)


def make_in_map(zhat_bf: np.ndarray, c: int) -> dict:
    """Build core c's transposed bf16 input: X blocks then Y blocks."""
    za, zp = zhat_bf[:HALF], zhat_bf[HALF:]
    x_is_anchor, xblocks, yblocks = _core_layout(c)
    xsrc, ysrc = (za, zp) if x_is_anchor else (zp, za)
    parts = [xsrc[b * Q : (b + 1) * Q] for b in xblocks]
    parts += [ysrc[b * Q : (b + 1) * Q] for b in yblocks]
    zc = np.ascontiguousarray(np.concatenate(parts, axis=0).T)
    return {"zc": zc}


def assemble(zhat_bf: np.ndarray, outs: list) -> np.float32:
    """Host-side O(N d) assembly of the per-core partials into the loss."""
    zf = zhat_bf.astype(np.float64)
    za, zp = zf[:HALF], zf[HALF:]
    Sa = np.zeros(HALF, dtype=np.float64)
    Sp = np.zeros(HALF, dtype=np.float64)
    for c in range(NCORES):
        o = outs[c]
        rows = np.asarray(o["rows"], dtype=np.float64)    # (128, 40)
        cols = np.asarray(o["cols"], dtype=np.float64)    # (10240,)
        x_is_anchor, xblocks, yblocks = _core_layout(c)
        SX, SY = (Sa, Sp) if x_is_anchor else (Sp, Sa)
        # rows[p, g*8+rc] belongs to center-block row rc*128+p
        r = rows.reshape(P, 5, 8).transpose(2, 0, 1).reshape(Q, 5)
        base = c * Q
        SX[base : base + Q] += r[:, 0] + r[:, 1] + r[:, 2]
        SY[base : base + Q] += r[:, 3] + r[:, 4]
        cols = cols.reshape(10, Q)
        for i, b in enumerate(xblocks[1:]):
            SX[b * Q : (b + 1) * Q] += cols[i]
        # y1 gets the pre-reduced rc0-5 partial plus raw rc6/rc7 chunks
        y1, y2, y3 = yblocks[1], yblocks[2], yblocks[3]
        SY[y2 * Q : (y2 + 1) * Q] += cols[5]
        SY[y3 * Q : (y3 + 1) * Q] += cols[6]
        etail = np.asarray(o["etail"], dtype=np.float64).reshape(2, P, Q)
        SY[y1 * Q : (y1 + 1) * Q] += (cols[4] + etail[0].sum(axis=0)
                                      + etail[1].sum(axis=0))
        # diag blocks' below-diagonal coverage (columns 128..1024)
        SX[base + 128 : base + Q] += cols[8][0:896]
        SY[base + 128 : base + Q] += cols[9][0:896]
    # self-similarity: the diagonal the device summed is sum_d bf16(zhat)^2
    # accumulated in f32 -- reproduce it (up to f32 summation order) here
    selfa = np.exp(np.sum(za * za, axis=1) * SCALE)
    selfp = np.exp(np.sum(zp * zp, axis=1) * SCALE)
    pdot = np.sum(za * zp, axis=1)
    terms = (np.log(Sa - selfa) + np.log(Sp - selfp) - pdot * SCALE)
    return np.float32(terms.mean())


_CACHE: dict = {}


def kernel(z, _trace: bool = False):
    z = np.ascontiguousarray(np.asarray(z, dtype=np.float32))
    assert z.shape == (M, D), z.shape
    if "nc" not in _CACHE:
        _CACHE["nc"] = build_kernel()
    nc = _CACHE["nc"]

    zhat_bf = _normalized_bf16(z)
    in_maps = [make_in_map(zhat_bf, c) for c in range(NCORES)]
    res = run_bass_kernel_spmd(
        nc, in_maps, core_ids=list(range(NCORES)), trace=_trace
    )
    _CACHE["last_results"] = res
    return assemble(zhat_bf, res.results)


# revision 51
# speedup vs baseline: 1.0988x; 1.0988x over previous
"""DHEL contrastive loss kernel for Trainium2 (8 NeuronCores, SPMD).

Math (reference):
  zhat = z / max(||z||, 1e-12) rowwise;  za = zhat[:8192], zp = zhat[8192:]
  sa_i = sum_j!=i exp(za_i . za_j / tau);  sp_i = sum_j!=i exp(zp_i . zp_j / tau)
  pdot_i = za_i . zp_i
  loss = mean_i( log sa_i + log sp_i - pdot_i / tau )

Key idea vs the naive row-parallel split: the two similarity matrices are
SYMMETRIC, so only the upper triangle of 1024x1024 blocks needs the (scalar-
engine-bound) exp evaluation. Each computed off-diagonal block (R, C)
contributes its row-sums to rows R (free via the activation accumulator) and
its column-sums to rows C (bf16 tree-add on DVE + a partition-reduce on the
otherwise idle Pool engine). Per-core exp work drops from 2*8192^2/8 = 16.8M
elements to 9 blocks = 9.4M, and the scalar engine runs wide (2048-col)
back-to-back exp+accumulate over the whole kernel.

Work assignment: a tournament orientation of K8 gives every core exactly 9
blocks: its own diagonal block in each half, its in-star pairs in one half and
out-star pairs in the other (4 + 3). Cores 0-3 get [5 anchor blocks, 4
positive], cores 4-7 the mirror image; the device program is identical
("X half" = 5 column blocks, "Y half" = 4) and the host maps halves/blocks
per core and inverts the mapping when assembling.

Division of labor (per the sharding hint, devices consume all-gathered
NORMALIZED embeddings): the host normalizes rows in f64, casts to bf16, and
ships each core the transposed [d, columns] slice it contracts against
(2.3 MB/core); block-granular DMAs so the first matmul issues ~4 us in. The
O(N^2 d) similarity/exp/reduction work all happens on device. The host folds
the returned row/column partial sums, subtracts the (exactly reproducible)
bf16 self-similarity, adds the positive-pair dots, and takes log+mean in f64
-- O(N d) assembly, 0.03% of the FLOPs.
"""

import sys

if "/opt/trn_rl_repo" not in sys.path:
    sys.path.insert(0, "/opt/trn_rl_repo")

from contextlib import ExitStack

import numpy as np

import concourse.bass as bass  # noqa: F401
import concourse.tile as tile
from concourse import bacc, mybir
from concourse.bass_utils import run_bass_kernel_spmd

P = 128
D = 128
M = 16384
HALF = M // 2       # 8192
Q = 1024            # rows per block
NCORES = 8
NT = 72             # 9 blocks x 8 tiles of 128 rows
TAU = 0.3
SCALE = float(1.0 / TAU)

# column groups: (lhs base col, rhs col offset, width, [colacc ids], tri)
# Local zt column blocks: [Cx, x1..x4, Cy, y1..y3] at 1024 cols each.
# colacc ids 0..3 are X partners x1..x4, 4..6 are Y partners y1..y3; diag
# slices have no block colacc. The Cy+y1 group runs LAST so the tail is
# short. tri=True groups start at the diagonal block: row-chunk rc only
# computes columns >= rc*128 (the block is symmetric, the lower triangle is
# recovered as column sums -> colout slots 8 (Cx) / 9 (Cy) cover block
# columns 128..1024 from the chunks above them).
GROUPS = [
    (0, 0, 1024, [], True),             # Cx (diag)
    (0, 1024, 2048, [0, 1], False),     # x1 x2
    (0, 3072, 2048, [2, 3], False),     # x3 x4
    (5120, 7168, 2048, [5, 6], False),  # y2 y3
    (5120, 5120, 2048, [-1, 4], True),  # Cy (diag) + y1
]
# DMA arrival order for the 9 column blocks (group consumption order).
DMA_ORDER = (0, 1, 2, 3, 4, 5, 7, 8, 6)

F32 = mybir.dt.float32
BF16 = mybir.dt.bfloat16
AF = mybir.ActivationFunctionType
OP = mybir.AluOpType
AX = mybir.AxisListType

# Bit-trick exp (Schraudolph) for the Pool-offloaded tail columns of groups
# 1-3: int32(s*EXPA + EXPB) read back as f32 bits approximates exp(s/tau);
# EXPB's bias is tuned so the error is mean-zero over sums (~1e-5 measured),
# and at most 3/16 of any row's sum comes from this path. The raw [128,512]
# results DMA to DRAM and the host folds their row/column partials, so the
# offload costs the DVE nothing and cannot pace the activation stream.
EXPA = float(np.float32(2.0**23 * SCALE * np.log2(np.e)))
EXPB = float(np.float32(127.0 * 2.0**23 - 476800.0))
OFF_W = 512
OFF_GROUPS = (1, 2, 3)


def _pair_owner(i: int, j: int) -> int:
    """Tournament owner of pair {i,j}, i<j: in-degrees (4,4,4,4,3,3,3,3)."""
    if j < 7:
        return i if (j - i) % 7 in (1, 2, 3) else j
    return i if i <= 3 else 7


def _core_layout(c: int):
    """Returns (x_is_anchor, xblocks, yblocks): block ids of the X (5-block)
    and Y (4-block) halves, center first."""
    a_star = []   # pairs {c,x} owned by c -> anchor-half partners
    p_star = []   # pairs not owned by c  -> positive-half partners
    for x in range(8):
        if x == c:
            continue
        i, j = min(c, x), max(c, x)
        (a_star if _pair_owner(i, j) == c else p_star).append(x)
    if len(a_star) == 4:
        return True, [c] + a_star, [c] + p_star
    return False, [c] + p_star, [c] + a_star


def _build(ctx: ExitStack, tc: tile.TileContext, zc_ext, rows_ext, cols_ext,
           etail_ext, eoff_ext):
    nc = tc.nc

    persist = ctx.enter_context(tc.tile_pool(name="persist", bufs=1))
    eo_pool = ctx.enter_context(tc.tile_pool(name="eo", bufs=8))
    psum_pool = ctx.enter_context(tc.tile_pool(name="psum", bufs=2,
                                               space="PSUM"))

    zt = persist.tile([P, NT * P], BF16, name="zt")
    rows = persist.tile([P, 5, 8], F32, name="rows")
    colacc = [persist.tile([P, Q], BF16, name=f"colacc{b}") for b in range(7)]
    colaccD = [persist.tile([P, 896], BF16, name=f"colaccD{m}")
               for m in range(2)]
    colout = persist.tile([1, 10, Q], F32, name="colout")
    off_pool = ctx.enter_context(tc.tile_pool(name="eoff", bufs=8))
    for k in DMA_ORDER:
        nc.sync.dma_start(zt[:, k * Q : (k + 1) * Q],
                          zc_ext[:, k * Q : (k + 1) * Q])

    def colsum_out(slot, src, off=0, width=Q, pool_dma=False):
        nc.gpsimd.tensor_reduce(colout[:, slot, off : off + width], src,
                                axis=AX.C, op=OP.add)
        eng = nc.gpsimd if pool_dma else nc.sync
        eng.dma_start(
            cols_ext[slot * Q + off : slot * Q + off + width].rearrange(
                "(o j) -> o j", o=1),
            colout[:, slot, off : off + width],
        )

    def sweep(g, posts=()):
        posts = list(posts)
        lhs0, off, w, caccs, tri = GROUPS[g]
        last = g == len(GROUPS) - 1
        offl = g in OFF_GROUPS
        goff = OFF_GROUPS.index(g) if offl else -1
        dacc = colaccD[0 if g == 0 else 1]
        for rc in range(8):
            cut = 128 * rc if tri else 0
            wr = w - cut
            aw = wr - OFF_W if offl else wr
            ps = psum_pool.tile([P, wr], F32, tag="ps", name=f"ps_g{g}_{rc}")
            cols_order = list(range(0, wr, 512))
            if offl:
                # offloaded chunk second: its Pool exp starts early but the
                # first ACT chunk is not delayed
                cols_order = cols_order[:1] + cols_order[-1:] + cols_order[1:-1]
            for col in cols_order:
                cw = min(512, wr - col)
                nc.tensor.matmul(
                    ps[:, col : col + cw],
                    zt[:, lhs0 + rc * P : lhs0 + (rc + 1) * P],
                    zt[:, off + cut + col : off + cut + col + cw],
                    start=True, stop=True,
                )
            if offl:
                eo2 = off_pool.tile([P, OFF_W], F32, tag="eo2",
                                    name=f"eo2_g{g}_{rc}")
                nc.vector.tensor_scalar(
                    eo2[:].bitcast(mybir.dt.int32), ps[:, aw:wr],
                    EXPA, EXPB, op0=OP.mult, op1=OP.add,
                )
                base = (goff * 8 + rc) * P
                nc.sync.dma_start(eoff_ext[base : base + P, :], eo2[:])
            eo = eo_pool.tile([P, aw], BF16, tag="eo", name=f"eo_g{g}_{rc}")
            nc.scalar.activation(
                eo[:], ps[:, 0:aw], AF.Exp, scale=SCALE,
                accum_out=rows[:, g, rc : rc + 1],
            )
            if tri and rc < 7:
                # below-diagonal coverage of the diag block: columns
                # [(rc+1)*128, 1024) accumulate as column sums
                sl = eo[:, 128 : Q - cut]
                dst = dacc[:, rc * 128 : 896]
                if rc == 0:
                    nc.vector.tensor_copy(dst, sl)
                else:
                    nc.vector.tensor_tensor(dst, dst, sl, op=OP.add)
            for i, b in enumerate(caccs):
                if b < 0:
                    continue
                sl = eo[:, i * Q - cut : min((i + 1) * Q - cut, aw)]
                cw = sl.shape[1]
                if last and rc >= 6:
                    # tail: ship the raw bf16 chunk to DRAM; the host sums
                    # the 128 partitions -- no post-activation engine work.
                    a = rc - 6
                    nc.scalar.dma_start(etail_ext[a * P : (a + 1) * P, :], sl)
                elif rc == 0:
                    nc.vector.tensor_copy(colacc[b][:, 0:cw], sl)
                else:
                    nc.vector.tensor_tensor(colacc[b][:, 0:cw],
                                            colacc[b][:, 0:cw], sl,
                                            op=OP.add)
            if rc >= 1 and (rc % 2 == 1 or not offl) and posts:
                posts.pop(0)()
            if last and rc == 5:
                # pre-reduce everything the tail depends on while chunks 6/7
                # still run: y1's rc0-5 partial and the diag columns below
                # 768 (rc6's add only touches 768..896)
                colsum_out(caccs[1], colacc[caccs[1]][:])
                colsum_out(9, dacc[:, 0:768], 0, 768)
            if last and rc == 6:
                colsum_out(9, dacc[:, 768:896], 768, 128)
        for t in posts:
            t()
        pending = []
        if not last:
            if tri:
                pending.append(lambda: colsum_out(8, dacc[:, 0:512], 0, 512))
                pending.append(lambda: colsum_out(8, dacc[:, 512:896],
                                                  512, 384))
            for b in caccs:
                if b >= 0:
                    # halves: smaller Pool quanta interleave with the next
                    # group's offload ts ops without head-of-line blocking
                    pending.append(
                        lambda b=b: colsum_out(b, colacc[b][:, 0:512],
                                               0, 512))
                    if not (offl and b == caccs[1]):
                        pending.append(
                            lambda b=b: colsum_out(b, colacc[b][:, 512:Q],
                                                   512, 512))
        return pending

    pending = []
    for g in range(len(GROUPS)):
        pending = sweep(g, pending)

    nc.scalar.dma_start(rows_ext.rearrange("p (g r) -> p g r", g=5),
                        rows[:])


def build_kernel() -> bass.Bass:
    nc = bacc.Bacc("TRN2", target_bir_lowering=False, debug=False,
                   num_devices=NCORES)
    zc_ext = nc.dram_tensor("zc", (D, NT * P), BF16, kind="ExternalInput").ap()
    rows_ext = nc.dram_tensor("rows", (P, 40), F32, kind="ExternalOutput").ap()
    # cols slots 0..6: colacc column sums (slot 4 = the tail block's rc0-5
    # partial); slots 8/9: the Cx/Cy diag blocks' below-diagonal column sums
    # (block columns 128..1024, width 896). Slot 7 is reserved/unused.
    cols_ext = nc.dram_tensor("cols", (10 * Q,), F32,
                              kind="ExternalOutput").ap()
    # raw bf16 eo chunks 6/7 of the tail block's y1 slice (host sums them)
    etail_ext = nc.dram_tensor("etail", (2 * P, Q), BF16,
                               kind="ExternalOutput").ap()
    # raw bit-trick exp results for the offloaded tail columns (f32 bits):
    # [group 1..3][rc 0..7] -> [128, 512]
    eoff_ext = nc.dram_tensor("eoff", (len(OFF_GROUPS) * 8 * P, OFF_W), F32,
                              kind="ExternalOutput").ap()
    with tile.TileContext(nc) as tc:
        with ExitStack() as ctx:
            _build(ctx, tc, zc_ext, rows_ext, cols_ext, etail_ext,
                   eoff_ext)
    nc.compile()
    return nc


def _normalized_bf16(z: np.ndarray) -> np.ndarray:
    import ml_dtypes

    zf = np.asarray(z, dtype=np.float64)
    zf = zf / np.maximum(np.linalg.norm(zf, axis=1, keepdims=True), 1e-12)
    return zf.astype(ml_dtypes.bfloat16)


def make_in_map(zhat_bf: np.ndarray, c: int) -> dict:
    """Build core c's transposed bf16 input: X blocks then Y blocks."""
    za, zp = zhat_bf[:HALF], zhat_bf[HALF:]
    x_is_anchor, xblocks, yblocks = _core_layout(c)
    xsrc, ysrc = (za, zp) if x_is_anchor else (zp, za)
    parts = [xsrc[b * Q : (b + 1) * Q] for b in xblocks]
    parts += [ysrc[b * Q : (b + 1) * Q] for b in yblocks]
    zc = np.ascontiguousarray(np.concatenate(parts, axis=0).T)
    return {"zc": zc}


def assemble(zhat_bf: np.ndarray, outs: list) -> np.float32:
    """Host-side O(N d) assembly of the per-core partials into the loss."""
    zf = zhat_bf.astype(np.float64)
    za, zp = zf[:HALF], zf[HALF:]
    Sa = np.zeros(HALF, dtype=np.float64)
    Sp = np.zeros(HALF, dtype=np.float64)
    for c in range(NCORES):
        o = outs[c]
        rows = np.asarray(o["rows"], dtype=np.float64)    # (128, 40)
        cols = np.asarray(o["cols"], dtype=np.float64)    # (10240,)
        x_is_anchor, xblocks, yblocks = _core_layout(c)
        SX, SY = (Sa, Sp) if x_is_anchor else (Sp, Sa)
        # rows[p, g*8+rc] belongs to center-block row rc*128+p
        r = rows.reshape(P, 5, 8).transpose(2, 0, 1).reshape(Q, 5)
        base = c * Q
        SX[base : base + Q] += r[:, 0] + r[:, 1] + r[:, 2]
        SY[base : base + Q] += r[:, 3] + r[:, 4]
        cols = cols.reshape(10, Q)
        # x2/x4/y3 had their last 512 columns computed on the Pool engine;
        # their device column sums cover only [0:512], the rest (and the
        # matching row partials) come from the raw eoff chunks below
        for i, b in enumerate(xblocks[1:]):
            if i in (1, 3):
                SX[b * Q : b * Q + 512] += cols[i][0:512]
            else:
                SX[b * Q : (b + 1) * Q] += cols[i]
        # y1 gets the pre-reduced rc0-5 partial plus raw rc6/rc7 chunks
        y1, y2, y3 = yblocks[1], yblocks[2], yblocks[3]
        SY[y2 * Q : (y2 + 1) * Q] += cols[5]
        SY[y3 * Q : y3 * Q + 512] += cols[6][0:512]
        etail = np.asarray(o["etail"], dtype=np.float64).reshape(2, P, Q)
        SY[y1 * Q : (y1 + 1) * Q] += (cols[4] + etail[0].sum(axis=0)
                                      + etail[1].sum(axis=0))
        eoff = np.asarray(o["eoff"], dtype=np.float64).reshape(3, 8, P, 512)
        SX[base : base + Q] += (eoff[0] + eoff[1]).sum(axis=2).reshape(Q)
        SY[base : base + Q] += eoff[2].sum(axis=2).reshape(Q)
        x2b, x4b = xblocks[2], xblocks[4]
        SX[x2b * Q + 512 : (x2b + 1) * Q] += eoff[0].sum(axis=(0, 1))
        SX[x4b * Q + 512 : (x4b + 1) * Q] += eoff[1].sum(axis=(0, 1))
        SY[y3 * Q + 512 : (y3 + 1) * Q] += eoff[2].sum(axis=(0, 1))
        # diag blocks' below-diagonal coverage (columns 128..1024)
        SX[base + 128 : base + Q] += cols[8][0:896]
        SY[base + 128 : base + Q] += cols[9][0:896]
    # self-similarity: the diagonal the device summed is sum_d bf16(zhat)^2
    # accumulated in f32 -- reproduce it (up to f32 summation order) here
    selfa = np.exp(np.sum(za * za, axis=1) * SCALE)
    selfp = np.exp(np.sum(zp * zp, axis=1) * SCALE)
    pdot = np.sum(za * zp, axis=1)
    terms = (np.log(Sa - selfa) + np.log(Sp - selfp) - pdot * SCALE)
    return np.float32(terms.mean())


_CACHE: dict = {}


def kernel(z, _trace: bool = False):
    z = np.ascontiguousarray(np.asarray(z, dtype=np.float32))
    assert z.shape == (M, D), z.shape
    if "nc" not in _CACHE:
        _CACHE["nc"] = build_kernel()
    nc = _CACHE["nc"]

    zhat_bf = _normalized_bf16(z)
    in_maps = [make_in_map(zhat_bf, c) for c in range(NCORES)]
    res = run_bass_kernel_spmd(
        nc, in_maps, core_ids=list(range(NCORES)), trace=_trace
    )
    _CACHE["last_results"] = res
    return assemble(zhat_bf, res.results)


# revision 54
# speedup vs baseline: 1.1027x; 1.0035x over previous
"""DHEL contrastive loss kernel for Trainium2 (8 NeuronCores, SPMD).

Math (reference):
  zhat = z / max(||z||, 1e-12) rowwise;  za = zhat[:8192], zp = zhat[8192:]
  sa_i = sum_j!=i exp(za_i . za_j / tau);  sp_i = sum_j!=i exp(zp_i . zp_j / tau)
  pdot_i = za_i . zp_i
  loss = mean_i( log sa_i + log sp_i - pdot_i / tau )

Key idea vs the naive row-parallel split: the two similarity matrices are
SYMMETRIC, so only the upper triangle of 1024x1024 blocks needs the (scalar-
engine-bound) exp evaluation. Each computed off-diagonal block (R, C)
contributes its row-sums to rows R (free via the activation accumulator) and
its column-sums to rows C (bf16 tree-add on DVE + a partition-reduce on the
otherwise idle Pool engine). Per-core exp work drops from 2*8192^2/8 = 16.8M
elements to 9 blocks = 9.4M, and the scalar engine runs wide (2048-col)
back-to-back exp+accumulate over the whole kernel.

Work assignment: a tournament orientation of K8 gives every core exactly 9
blocks: its own diagonal block in each half, its in-star pairs in one half and
out-star pairs in the other (4 + 3). Cores 0-3 get [5 anchor blocks, 4
positive], cores 4-7 the mirror image; the device program is identical
("X half" = 5 column blocks, "Y half" = 4) and the host maps halves/blocks
per core and inverts the mapping when assembling.

Division of labor (per the sharding hint, devices consume all-gathered
NORMALIZED embeddings): the host normalizes rows in f64, casts to bf16, and
ships each core the transposed [d, columns] slice it contracts against
(2.3 MB/core); block-granular DMAs so the first matmul issues ~4 us in. The
O(N^2 d) similarity/exp/reduction work all happens on device. The host folds
the returned row/column partial sums, subtracts the (exactly reproducible)
bf16 self-similarity, adds the positive-pair dots, and takes log+mean in f64
-- O(N d) assembly, 0.03% of the FLOPs.
"""

import sys

if "/opt/trn_rl_repo" not in sys.path:
    sys.path.insert(0, "/opt/trn_rl_repo")

from contextlib import ExitStack

import numpy as np

import concourse.bass as bass  # noqa: F401
import concourse.tile as tile
from concourse import bacc, mybir
from concourse.bass_utils import run_bass_kernel_spmd

P = 128
D = 128
M = 16384
HALF = M // 2       # 8192
Q = 1024            # rows per block
NCORES = 8
NT = 72             # 9 blocks x 8 tiles of 128 rows
TAU = 0.3
SCALE = float(1.0 / TAU)

# column groups: (lhs base col, rhs col offset, width, [colacc ids], tri)
# Local zt column blocks: [Cx, x1..x4, Cy, y1..y3] at 1024 cols each.
# colacc ids 0..3 are X partners x1..x4, 4..6 are Y partners y1..y3; diag
# slices have no block colacc. The Cy+y1 group runs LAST so the tail is
# short. tri=True groups start at the diagonal block: row-chunk rc only
# computes columns >= rc*128 (the block is symmetric, the lower triangle is
# recovered as column sums -> colout slots 8 (Cx) / 9 (Cy) cover block
# columns 128..1024 from the chunks above them).
GROUPS = [
    (0, 0, 1024, [], True),             # Cx (diag)
    (0, 1024, 2048, [0, 1], False),     # x1 x2
    (0, 3072, 2048, [2, 3], False),     # x3 x4
    (5120, 7168, 2048, [5, 6], False),  # y2 y3
    (5120, 5120, 2048, [-1, 4], True),  # Cy (diag) + y1
]
# DMA arrival order for the 9 column blocks (group consumption order).
DMA_ORDER = (0, 1, 2, 3, 4, 5, 7, 8, 6)

F32 = mybir.dt.float32
BF16 = mybir.dt.bfloat16
AF = mybir.ActivationFunctionType
OP = mybir.AluOpType
AX = mybir.AxisListType

# Bit-trick exp (Schraudolph) for the Pool-offloaded tail columns of groups
# 1-3: int32(s*EXPA + EXPB) read back as f32 bits approximates exp(s/tau);
# EXPB's bias is tuned so the error is mean-zero over sums (~1e-5 measured),
# and at most 3/16 of any row's sum comes from this path. The raw [128,512]
# results DMA to DRAM and the host folds their row/column partials, so the
# offload costs the DVE nothing and cannot pace the activation stream.
EXPA = float(np.float32(2.0**23 * SCALE * np.log2(np.e)))
EXPB = float(np.float32(127.0 * 2.0**23 - 476800.0))
OFF_W = 512
OFF_GROUPS = (1, 2, 3)


def _pair_owner(i: int, j: int) -> int:
    """Tournament owner of pair {i,j}, i<j: in-degrees (4,4,4,4,3,3,3,3)."""
    if j < 7:
        return i if (j - i) % 7 in (1, 2, 3) else j
    return i if i <= 3 else 7


def _core_layout(c: int):
    """Returns (x_is_anchor, xblocks, yblocks): block ids of the X (5-block)
    and Y (4-block) halves, center first."""
    a_star = []   # pairs {c,x} owned by c -> anchor-half partners
    p_star = []   # pairs not owned by c  -> positive-half partners
    for x in range(8):
        if x == c:
            continue
        i, j = min(c, x), max(c, x)
        (a_star if _pair_owner(i, j) == c else p_star).append(x)
    if len(a_star) == 4:
        return True, [c] + a_star, [c] + p_star
    return False, [c] + p_star, [c] + a_star


def _build(ctx: ExitStack, tc: tile.TileContext, zc_ext, rows_ext, cols_ext,
           etail_ext, eoff_ext):
    nc = tc.nc

    persist = ctx.enter_context(tc.tile_pool(name="persist", bufs=1))
    eo_pool = ctx.enter_context(tc.tile_pool(name="eo", bufs=12))
    psum_pool = ctx.enter_context(tc.tile_pool(name="psum", bufs=2,
                                               space="PSUM"))

    zt = persist.tile([P, NT * P], BF16, name="zt")
    rows = persist.tile([P, 5, 8], F32, name="rows")
    rampacc = persist.tile([P, 1], F32, name="rampacc")
    colacc = [persist.tile([P, Q], BF16, name=f"colacc{b}") for b in range(7)]
    colaccD = [persist.tile([P, 896], BF16, name=f"colaccD{m}")
               for m in range(2)]
    colout = persist.tile([1, 10, Q], F32, name="colout")
    off_pool = ctx.enter_context(tc.tile_pool(name="eoff", bufs=12))
    # block 0 lands in two halves so the ramp's first matmul starts early
    nc.sync.dma_start(zt[:, 0:512], zc_ext[:, 0:512])
    nc.sync.dma_start(zt[:, 512:Q], zc_ext[:, 512:Q])
    for k in DMA_ORDER[1:]:
        nc.sync.dma_start(zt[:, k * Q : (k + 1) * Q],
                          zc_ext[:, k * Q : (k + 1) * Q])

    def colsum_out(slot, src, off=0, width=Q, pool_dma=False):
        nc.gpsimd.tensor_reduce(colout[:, slot, off : off + width], src,
                                axis=AX.C, op=OP.add)
        eng = nc.gpsimd if pool_dma else nc.sync
        eng.dma_start(
            cols_ext[slot * Q + off : slot * Q + off + width].rearrange(
                "(o j) -> o j", o=1),
            colout[:, slot, off : off + width],
        )

    def sweep(g, posts=()):
        posts = list(posts)
        lhs0, off, w, caccs, tri = GROUPS[g]
        last = g == len(GROUPS) - 1
        offl = g in OFF_GROUPS
        goff = OFF_GROUPS.index(g) if offl else -1
        dacc = colaccD[0 if g == 0 else 1]
        for rc in range(8):
            cut = 128 * rc if tri else 0
            wr = w - cut
            if g == 0 and rc == 0:
                # ramp: two 512-wide calls; the first runs as soon as the
                # first half-block of zc lands
                ps = psum_pool.tile([P, Q], F32, tag="ps", name="ps_ramp")
                eo_a = eo_pool.tile([P, 512], BF16, tag="eo", name="eo_r0a")
                eo_b = eo_pool.tile([P, 512], BF16, tag="eo", name="eo_r0b")
                nc.tensor.matmul(ps[:, 0:512], zt[:, 0:P], zt[:, 0:512],
                                 start=True, stop=True)
                nc.scalar.activation(eo_a[:], ps[:, 0:512], AF.Exp,
                                     scale=SCALE,
                                     accum_out=rows[:, 0, 0:1])
                nc.tensor.matmul(ps[:, 512:Q], zt[:, 0:P], zt[:, 512:Q],
                                 start=True, stop=True)
                nc.scalar.activation(eo_b[:], ps[:, 512:Q], AF.Exp,
                                     scale=SCALE, accum_out=rampacc[:, 0:1])
                nc.vector.tensor_copy(dacc[:, 0:384], eo_a[:, 128:512])
                nc.vector.tensor_copy(dacc[:, 384:896], eo_b[:])
                continue
            aw = wr - OFF_W if offl else wr
            ps = psum_pool.tile([P, wr], F32, tag="ps", name=f"ps_g{g}_{rc}")
            cols_order = list(range(0, wr, 512))
            if offl:
                # offloaded chunk second: its Pool exp starts early but the
                # first ACT chunk is not delayed
                cols_order = cols_order[:1] + cols_order[-1:] + cols_order[1:-1]
            for col in cols_order:
                cw = min(512, wr - col)
                nc.tensor.matmul(
                    ps[:, col : col + cw],
                    zt[:, lhs0 + rc * P : lhs0 + (rc + 1) * P],
                    zt[:, off + cut + col : off + cut + col + cw],
                    start=True, stop=True,
                )
            if offl:
                eo2 = off_pool.tile([P, OFF_W], F32, tag="eo2",
                                    name=f"eo2_g{g}_{rc}")
                nc.vector.tensor_scalar(
                    eo2[:].bitcast(mybir.dt.int32), ps[:, aw:wr],
                    EXPA, EXPB, op0=OP.mult, op1=OP.add,
                )
                base = (goff * 8 + rc) * P
                nc.sync.dma_start(eoff_ext[base : base + P, :], eo2[:])
            eo = eo_pool.tile([P, aw], BF16, tag="eo", name=f"eo_g{g}_{rc}")
            nc.scalar.activation(
                eo[:], ps[:, 0:aw], AF.Exp, scale=SCALE,
                accum_out=rows[:, g, rc : rc + 1],
            )
            if tri and rc < 7:
                # below-diagonal coverage of the diag block: columns
                # [(rc+1)*128, 1024) accumulate as column sums
                sl = eo[:, 128 : Q - cut]
                dst = dacc[:, rc * 128 : 896]
                if rc == 0:
                    nc.vector.tensor_copy(dst, sl)
                else:
                    nc.vector.tensor_tensor(dst, dst, sl, op=OP.add)
            for i, b in enumerate(caccs):
                if b < 0:
                    continue
                sl = eo[:, i * Q - cut : min((i + 1) * Q - cut, aw)]
                cw = sl.shape[1]
                if last and rc >= 6:
                    # tail: ship the raw bf16 chunk to DRAM; the host sums
                    # the 128 partitions -- no post-activation engine work.
                    a = rc - 6
                    nc.scalar.dma_start(etail_ext[a * P : (a + 1) * P, :], sl)
                elif rc == 0:
                    eng = nc.gpsimd if offl and b == caccs[1] else nc.vector
                    eng.tensor_copy(colacc[b][:, 0:cw], sl)
                else:
                    # the offloaded groups' half-width accumulator runs on
                    # Pool (1.1us/add < the 1.8us rc wall) to keep DVE off
                    # the psum-release critical path
                    eng = nc.gpsimd if offl and b == caccs[1] else nc.vector
                    eng.tensor_tensor(colacc[b][:, 0:cw],
                                      colacc[b][:, 0:cw], sl,
                                      op=OP.add)
            if rc >= 1 and (rc % 2 == 1 or not offl) and posts:
                posts.pop(0)()
            if last and rc == 5:
                # pre-reduce everything the tail depends on while chunks 6/7
                # still run: y1's rc0-5 partial and the diag columns below
                # 768 (rc6's add only touches 768..896)
                colsum_out(caccs[1], colacc[caccs[1]][:])
                colsum_out(9, dacc[:, 0:768], 0, 768)
            if last and rc == 6:
                colsum_out(9, dacc[:, 768:896], 768, 128)
        for t in posts:
            t()
        pending = []
        if not last:
            if tri:
                pending.append(lambda: colsum_out(8, dacc[:, 0:512], 0, 512))
                pending.append(lambda: colsum_out(8, dacc[:, 512:896],
                                                  512, 384))
            for b in caccs:
                if b >= 0:
                    # halves: smaller Pool quanta interleave with the next
                    # group's offload ts ops without head-of-line blocking
                    pending.append(
                        lambda b=b: colsum_out(b, colacc[b][:, 0:512],
                                               0, 512))
                    if not (offl and b == caccs[1]):
                        pending.append(
                            lambda b=b: colsum_out(b, colacc[b][:, 512:Q],
                                                   512, 512))
        return pending

    pending = []
    for g in range(len(GROUPS)):
        pending = sweep(g, pending)

    nc.scalar.dma_start(
        rows_ext[:, 0:40].rearrange("p (g r) -> p g r", g=5), rows[:])
    nc.scalar.dma_start(
        rows_ext[:, 40:41].rearrange("p o -> p o"), rampacc[:])


def build_kernel() -> bass.Bass:
    nc = bacc.Bacc("TRN2", target_bir_lowering=False, debug=False,
                   num_devices=NCORES)
    zc_ext = nc.dram_tensor("zc", (D, NT * P), BF16, kind="ExternalInput").ap()
    rows_ext = nc.dram_tensor("rows", (P, 41), F32, kind="ExternalOutput").ap()
    # cols slots 0..6: colacc column sums (slot 4 = the tail block's rc0-5
    # partial); slots 8/9: the Cx/Cy diag blocks' below-diagonal column sums
    # (block columns 128..1024, width 896). Slot 7 is reserved/unused.
    cols_ext = nc.dram_tensor("cols", (10 * Q,), F32,
                              kind="ExternalOutput").ap()
    # raw bf16 eo chunks 6/7 of the tail block's y1 slice (host sums them)
    etail_ext = nc.dram_tensor("etail", (2 * P, Q), BF16,
                               kind="ExternalOutput").ap()
    # raw bit-trick exp results for the offloaded tail columns (f32 bits):
    # [group 1..3][rc 0..7] -> [128, 512]
    eoff_ext = nc.dram_tensor("eoff", (len(OFF_GROUPS) * 8 * P, OFF_W), F32,
                              kind="ExternalOutput").ap()
    with tile.TileContext(nc) as tc:
        with ExitStack() as ctx:
            _build(ctx, tc, zc_ext, rows_ext, cols_ext, etail_ext,
                   eoff_ext)
    nc.compile()
    return nc


def _normalized_bf16(z: np.ndarray) -> np.ndarray:
    import ml_dtypes

    zf = np.asarray(z, dtype=np.float64)
    zf = zf / np.maximum(np.linalg.norm(zf, axis=1, keepdims=True), 1e-12)
    return zf.astype(ml_dtypes.bfloat16)


def make_in_map(zhat_bf: np.ndarray, c: int) -> dict:
    """Build core c's transposed bf16 input: X blocks then Y blocks."""
    za, zp = zhat_bf[:HALF], zhat_bf[HALF:]
    x_is_anchor, xblocks, yblocks = _core_layout(c)
    xsrc, ysrc = (za, zp) if x_is_anchor else (zp, za)
    parts = [xsrc[b * Q : (b + 1) * Q] for b in xblocks]
    parts += [ysrc[b * Q : (b + 1) * Q] for b in yblocks]
    zc = np.ascontiguousarray(np.concatenate(parts, axis=0).T)
    return {"zc": zc}


def assemble(zhat_bf: np.ndarray, outs: list) -> np.float32:
    """Host-side O(N d) assembly of the per-core partials into the loss."""
    zf = zhat_bf.astype(np.float64)
    za, zp = zf[:HALF], zf[HALF:]
    Sa = np.zeros(HALF, dtype=np.float64)
    Sp = np.zeros(HALF, dtype=np.float64)
    for c in range(NCORES):
        o = outs[c]
        rows = np.asarray(o["rows"], dtype=np.float64)    # (128, 41)
        cols = np.asarray(o["cols"], dtype=np.float64)    # (10240,)
        x_is_anchor, xblocks, yblocks = _core_layout(c)
        SX, SY = (Sa, Sp) if x_is_anchor else (Sp, Sa)
        # rows[p, g*8+rc] belongs to center-block row rc*128+p
        r = rows[:, 0:40].reshape(P, 5, 8).transpose(2, 0, 1).reshape(Q, 5)
        base = c * Q
        SX[base : base + Q] += r[:, 0] + r[:, 1] + r[:, 2]
        # G0 rc0's split second call accumulated separately (rows 0..127)
        SX[base : base + P] += rows[:, 40]
        SY[base : base + Q] += r[:, 3] + r[:, 4]
        cols = cols.reshape(10, Q)
        # x2/x4/y3 had their last 512 columns computed on the Pool engine;
        # their device column sums cover only [0:512], the rest (and the
        # matching row partials) come from the raw eoff chunks below
        for i, b in enumerate(xblocks[1:]):
            if i in (1, 3):
                SX[b * Q : b * Q + 512] += cols[i][0:512]
            else:
                SX[b * Q : (b + 1) * Q] += cols[i]
        # y1 gets the pre-reduced rc0-5 partial plus raw rc6/rc7 chunks
        y1, y2, y3 = yblocks[1], yblocks[2], yblocks[3]
        SY[y2 * Q : (y2 + 1) * Q] += cols[5]
        SY[y3 * Q : y3 * Q + 512] += cols[6][0:512]
        etail = np.asarray(o["etail"], dtype=np.float64).reshape(2, P, Q)
        SY[y1 * Q : (y1 + 1) * Q] += (cols[4] + etail[0].sum(axis=0)
                                      + etail[1].sum(axis=0))
        eoff = np.asarray(o["eoff"], dtype=np.float64).reshape(3, 8, P, 512)
        SX[base : base + Q] += (eoff[0] + eoff[1]).sum(axis=2).reshape(Q)
        SY[base : base + Q] += eoff[2].sum(axis=2).reshape(Q)
        x2b, x4b = xblocks[2], xblocks[4]
        SX[x2b * Q + 512 : (x2b + 1) * Q] += eoff[0].sum(axis=(0, 1))
        SX[x4b * Q + 512 : (x4b + 1) * Q] += eoff[1].sum(axis=(0, 1))
        SY[y3 * Q + 512 : (y3 + 1) * Q] += eoff[2].sum(axis=(0, 1))
        # diag blocks' below-diagonal coverage (columns 128..1024)
        SX[base + 128 : base + Q] += cols[8][0:896]
        SY[base + 128 : base + Q] += cols[9][0:896]
    # self-similarity: the diagonal the device summed is sum_d bf16(zhat)^2
    # accumulated in f32 -- reproduce it (up to f32 summation order) here
    selfa = np.exp(np.sum(za * za, axis=1) * SCALE)
    selfp = np.exp(np.sum(zp * zp, axis=1) * SCALE)
    pdot = np.sum(za * zp, axis=1)
    terms = (np.log(Sa - selfa) + np.log(Sp - selfp) - pdot * SCALE)
    return np.float32(terms.mean())


_CACHE: dict = {}


def kernel(z, _trace: bool = False):
    z = np.ascontiguousarray(np.asarray(z, dtype=np.float32))
    assert z.shape == (M, D), z.shape
    if "nc" not in _CACHE:
        _CACHE["nc"] = build_kernel()
    nc = _CACHE["nc"]

    zhat_bf = _normalized_bf16(z)
    in_maps = [make_in_map(zhat_bf, c) for c in range(NCORES)]
    res = run_bass_kernel_spmd(
        nc, in_maps, core_ids=list(range(NCORES)), trace=_trace
    )
    _CACHE["last_results"] = res
    return assemble(zhat_bf, res.results)
